# revision 1
# baseline (speedup 1.0000x reference)
"""Trainium2 Bass kernel for nn_JointRelationModule (self-contained).

Math (per person p; softmax is segment-softmax over persons within an imgid
group, elementwise over the (K,K) score entries):
    q = Wq x + bq ; k = Wk x + bk ; v = Wv x + bv      (1x1 conv over K=17)
    S_p = q_p k_p^T / 64
    attn = segment-softmax over persons
    out = relu(attn_p @ v_p + x_p)

Device formulation (heavy ops bf16 on the PE, block-column layouts):
  - Stack BD=7 persons as [119, hw]. Per stack: G = x x^T via PE transpose +
    accumulating matmuls (bf16, f32 PSUM).
  - scores^T in block-column layout [119, 17] via a masked-Gram matmul chain
    (block-diag mask kills cross-person terms), so no gather/scatter DMAs.
  - Segment softmax via per-stack selector matmuls into group-slot tiles,
    reciprocal, selector-transpose broadcast back; all partition-aligned.
  - Output: B = blockdiag((attn Wv)^T) + I with an av row appended; the
    residual and v-bias ride along x_aug (all-ones row), so each output chunk
    is one matmul + one relu. Stored bf16, host upcasts.

Data movement: x and y live in a partition-major layout [120, S*hw] so a
multi-stack tile is one DMA with 16KB-contiguous per-partition descriptors
(per-queue DMA throughput here is descriptor-rate-limited); every load/store
is split across the three DMA-capable queues (sync/gpsimd/scalar) by
partition range. Loads are emitted just-in-time with the compute emission.

Sharding: data-parallel over persons at imgid group boundaries (8 cores),
weights replicated. Host casts x to bf16 (halves load bytes); output comes
back bf16 (halves store bytes). Tolerance 2e-2; measured error ~5e-3.
"""

import math
import sys

import numpy as np

K = 17
HW = 4096  # 64*64
P_TOTAL = 512
N_CORES = 8
NORM = 64.0
BD = 7          # persons per stack
BDK = BD * K    # 119
O_CH = 512      # output chunk cols (one PSUM bank of f32)

_cache: dict = {}


def _ensure_path():
    try:
        import concourse.bass  # noqa: F401
    except ImportError:
        for p in ("/opt/trn_rl_repo", "/root/.axon_site/_ro/trn_rl_repo"):
            if p not in sys.path:
                sys.path.insert(0, p)
        import concourse.bass  # noqa: F401


def _build(S0: int, S1: int, T0: int, T: int, have_bias: bool, used: tuple):
    """Builds + compiles the per-core SPMD Bass program.

    Persons are split into two independent softmax halves (stacks [0,S0) with
    group-slot tiles [0,T0), stacks [S0,S) with tiles [T0,T)) so half-1's
    loads/grams overlap half-0's outputs/stores."""
    _ensure_path()
    import concourse.bacc as bacc
    import concourse.mybir as mybir
    import concourse.tile as tile

    f32 = mybir.dt.float32
    bf16 = mybir.dt.bfloat16
    Exp = mybir.ActivationFunctionType.Exp
    Relu = mybir.ActivationFunctionType.Relu

    S = S0 + S1
    U = len(used)

    nc = bacc.Bacc(
        "TRN2",
        target_bir_lowering=False,
        debug=False,
        enable_asserts=False,
        num_devices=N_CORES,
    )

    x_d = nc.dram_tensor("x", [BDK + 1, S * HW], bf16, kind="ExternalInput")
    wq_d = nc.dram_tensor("wq_col", [BDK, K], f32, kind="ExternalInput")
    wk_d = nc.dram_tensor("wkt_bd", [BDK, BDK], f32, kind="ExternalInput")
    wv_d = nc.dram_tensor("wv_aug", [BDK, BDK + 1], bf16, kind="ExternalInput")
    id_d = nc.dram_tensor("id119", [BDK, BDK], bf16, kind="ExternalInput")
    ia_d = nc.dram_tensor("iaug", [BDK + 1, BDK], f32, kind="ExternalInput")
    mk_d = nc.dram_tensor("bdmask", [BDK, BDK], f32, kind="ExternalInput")
    mkb_d = nc.dram_tensor("bdmaskb", [BDK, BDK], bf16, kind="ExternalInput")
    sel_d = nc.dram_tensor("sel", [BDK, U * BDK], bf16, kind="ExternalInput")
    selt_d = nc.dram_tensor("selT", [BDK, U * BDK], bf16,
                            kind="ExternalInput")
    if have_bias:
        corr_d = nc.dram_tensor("corr_col", [BDK, K * S], f32,
                                kind="ExternalInput")
    y_d = nc.dram_tensor("y", [BDK, S * HW], bf16, kind="ExternalOutput")

    with tile.TileContext(nc) as tc:
        with (
            nc.allow_low_precision(reason="bf16 softmax ok at 2e-2 tol"),
            tc.tile_pool(name="xpool", bufs=1) as xpool,
            tc.tile_pool(name="cpool", bufs=1) as cpool,
            tc.tile_pool(name="wpool", bufs=2) as wpool,
            tc.tile_pool(name="opool", bufs=2) as opool,
            tc.tile_pool(name="pp", bufs=2, space="PSUM") as pp,
        ):
            queues = (nc.sync, nc.gpsimd, nc.scalar)
            PSPLIT = (slice(0, 40), slice(40, 80), slice(80, BDK + 1))
            PSPLIT_Y = (slice(0, 40), slice(40, 80), slice(80, BDK))

            # --- tiny phase-A constants first (id_t gates every transpose) ---
            id_t = cpool.tile([BDK, BDK], bf16, name="id_t", tag="id")
            mk_t = cpool.tile([BDK, BDK], f32, name="mk_t", tag="mk")
            wq_t = cpool.tile([BDK, K], f32, name="wq_t", tag="wq")
            wk_t = cpool.tile([BDK, BDK], f32, name="wk_t", tag="wk")
            nc.sync.dma_start(id_t[:], id_d.ap())
            nc.gpsimd.dma_start(mk_t[:], mk_d.ap())
            nc.sync.dma_start(wq_t[:], wq_d.ap())
            nc.gpsimd.dma_start(wk_t[:], wk_d.ap())

            x_tiles = []  # per stack

            def load_xtile(s):
                xt_ = xpool.tile([BDK + 1, HW], bf16, name=f"xp{s}",
                                 tag=f"xp{s}")
                csl = slice(s * HW, (s + 1) * HW)
                for qi, psl in enumerate(PSPLIT):
                    queues[qi].dma_start(xt_[psl, :], x_d.ap()[psl, csl])
                x_tiles.append(xt_)

            load_xtile(0)
            if have_bias:
                corr_t = cpool.tile([BDK, K * S], f32, name="corr_t",
                                    tag="corr")
                nc.scalar.dma_start(corr_t[:], corr_d.ap())

            # bulkier constants: tiles declared now, DMAs emitted mid-phase-A
            # (the framework coalesces DMA waits into a cumulative counter, so
            # anything emitted before the first transpose delays it)
            wv_t = cpool.tile([BDK, BDK + 1], bf16, name="wv_t", tag="wv")
            ia_t = cpool.tile([BDK + 1, BDK], f32, name="ia_t", tag="ia")
            sel_t = cpool.tile([BDK, U * BDK], bf16, name="sel_t", tag="sel")
            selt_t = cpool.tile([BDK, U * BDK], bf16, name="selt_t",
                                tag="selt")
            mkb_t = cpool.tile([BDK, BDK], bf16, name="mkb_t", tag="mkb")

            def emit_const_dmas():
                nc.sync.dma_start(wv_t[:], wv_d.ap())
                nc.gpsimd.dma_start(ia_t[:], ia_d.ap())
                nc.scalar.dma_start(mkb_t[:], mkb_d.ap())
                nc.sync.dma_start(sel_t[:], sel_d.ap())
                nc.gpsimd.dma_start(selt_t[:], selt_d.ap())

            exp_all = cpool.tile([BDK, K * S], bf16, name="exp_all", tag="exp")
            inv_t = cpool.tile([BDK, K * T], bf16, name="inv_t", tag="inv")

            # --- phase A: transpose -> gram -> scores^T -> exp, skewed ---
            # PSUM tags (8 banks): big=tp/o_ps x4, gsb=g/seg/b x2, tiny x2
            G_CH = 1024          # x cols per transpose group
            n_grp = HW // G_CH   # 4 groups per stack
            TC = BDK + 1         # 120: chunk col stride (4B-aligned in PSUM)
            state = {"ncopy": 0, "loaded": 1}
            g_tiles = {}

            def emit_transposes(s, gi):
                xt_ = x_tiles[s]
                tp = pp.tile([128, 8 * TC], bf16, name="tp", tag="big",
                             bufs=4)
                for c8 in range(8):
                    col = G_CH * gi + 128 * c8
                    nc.tensor.transpose(
                        tp[:, TC * c8:TC * c8 + BDK],
                        xt_[0:BDK, col:col + 128], id_t[:],
                    )
                xt = wpool.tile([128, 8 * TC], bf16, name="xt", tag="xt",
                                bufs=4)
                if state["ncopy"] % 2 == 0:
                    nc.vector.tensor_copy(xt[:], tp[:])
                else:
                    nc.scalar.copy(xt[:], tp[:])
                state["ncopy"] += 1
                return xt

            def emit_gram(s, gi, xt):
                if s not in g_tiles:
                    g_tiles[s] = pp.tile([BDK + 1, BDK], f32, name=f"g{s}",
                                         tag="gsb", bufs=2)
                g_ps = g_tiles[s]
                for c8 in range(8):
                    nc.tensor.matmul(
                        g_ps[0:BDK, :], xt[:, TC * c8:TC * c8 + BDK],
                        xt[:, TC * c8:TC * c8 + BDK],
                        start=(gi == 0 and c8 == 0),
                        stop=(gi == n_grp - 1 and c8 == 7),
                    )
                if gi == n_grp - 1:
                    emit_tiny_chain(s)

            def emit_tiny_chain(s):
                g_sb = wpool.tile([BDK, BDK], f32, name="g_sb", tag="g_sb",
                                  bufs=2)
                nc.vector.tensor_mul(g_sb[:], g_tiles[s][0:BDK, :], mk_t[:])
                m1_ps = pp.tile([BDK, K], f32, name="m1", tag="tiny", bufs=2)
                nc.tensor.matmul(m1_ps[:], g_sb[:], wq_t[:], start=True,
                                 stop=True)
                m1_sb = wpool.tile([BDK, K], f32, name="m1_sb", tag="m1_sb",
                                   bufs=2)
                nc.scalar.copy(m1_sb[:], m1_ps[:])
                st_ps = pp.tile([BDK, K], f32, name="st", tag="tiny", bufs=2)
                nc.tensor.matmul(st_ps[:], wk_t[:], m1_sb[:], start=True,
                                 stop=True)
                esl = slice(K * s, K * (s + 1))
                if have_bias:
                    eb_sb = wpool.tile([BDK, K], f32, name="eb_sb", tag="eb")
                    nc.vector.tensor_add(eb_sb[:], st_ps[:], corr_t[:, esl])
                    nc.scalar.activation(exp_all[:, esl], eb_sb[:], Exp)
                else:
                    nc.scalar.activation(exp_all[:, esl], st_ps[:], Exp)

            pend = []

            def emit_A_stack(s):
                if s == 2:
                    emit_const_dmas()
                for gi in range(n_grp):
                    pend.append((s, gi, emit_transposes(s, gi)))
                    # just-in-time prefetch, interleaved with compute emission
                    while state["loaded"] < min(s + 3, S):
                        load_xtile(state["loaded"])
                        state["loaded"] += 1
                    if len(pend) > 2:
                        ps, pgi, xt = pend.pop(0)
                        emit_gram(ps, pgi, xt)

            def drain_pend():
                while pend:
                    ps, pgi, xt = pend.pop(0)
                    emit_gram(ps, pgi, xt)

            # --- phase C (per half): segment sums -> reciprocal ---
            def emit_phase_C(trange):
                seg_tiles = {}
                for t in trange:
                    idxs = [i for i, (ss, tt) in enumerate(used) if tt == t]
                    seg_ps = pp.tile([BDK + 1, BDK], f32, name=f"seg{t}",
                                     tag="gsb", bufs=2)
                    for n, i in enumerate(idxs):
                        s = used[i][0]
                        nc.tensor.matmul(
                            seg_ps[0:BDK, 0:K],
                            sel_t[:, BDK * i:BDK * (i + 1)],
                            exp_all[:, K * s:K * (s + 1)],
                            start=(n == 0), stop=(n == len(idxs) - 1),
                        )
                    seg_tiles[t] = seg_ps
                for t in trange:
                    seg_sb = wpool.tile([BDK, K], f32, name="seg_sb",
                                        tag="seg_sb")
                    nc.vector.tensor_scalar_max(
                        seg_sb[:], seg_tiles[t][0:BDK, 0:K], 1e-30)
                    nc.vector.reciprocal(inv_t[:, K * t:K * (t + 1)],
                                         seg_sb[:])

            # --- phase D: pipelined per stack ---
            state["nrelu"] = 0
            attn_tiles = {}

            def emit_attn_chain(s):
                idxs = [i for i, (ss, tt) in enumerate(used) if ss == s]
                invb_ps = pp.tile([BDK, K], f32, name="invb", tag="tiny",
                                  bufs=2)
                for n, i in enumerate(idxs):
                    t = used[i][1]
                    nc.tensor.matmul(
                        invb_ps[:],
                        selt_t[:, BDK * i:BDK * (i + 1)],
                        inv_t[:, K * t:K * (t + 1)],
                        start=(n == 0), stop=(n == len(idxs) - 1),
                    )
                attn_sb = wpool.tile([BDK, K], bf16, name="attn_sb",
                                     tag="attn_c", bufs=2)
                nc.vector.tensor_mul(attn_sb[:], exp_all[:, K * s:K * (s + 1)],
                                     invb_ps[:])
                attn_bd = wpool.tile([BDK, BDK], bf16, name="attn_bd",
                                     tag="attn", bufs=2)
                for j in range(BD):
                    jsl = slice(K * j, K * (j + 1))
                    nc.gpsimd.tensor_mul(attn_bd[:, jsl], attn_sb[:],
                                         mkb_t[:, jsl])
                attn_tiles[s] = attn_bd

            def emit_D_stack(s):
                b_ps = pp.tile([BDK + 1, BDK], f32, name="b_ps", tag="gsb",
                               bufs=2)
                nc.tensor.matmul(b_ps[:], wv_t[:], attn_tiles.pop(s)[:],
                                 start=True, stop=True)
                b_sb = wpool.tile([BDK + 1, BDK], bf16, name="b_sb", tag="B",
                                  bufs=2)
                nc.vector.tensor_add(b_sb[:], b_ps[:], ia_t[:])
                if s + 1 < S and s + 1 != S0:
                    emit_attn_chain(s + 1)  # overlaps this stack's matmuls

                osb = opool.tile([BDK, HW], bf16, name="osb", tag="osb",
                                 bufs=3)
                xt_ = x_tiles[s]
                for oc in range(HW // O_CH):
                    sl = slice(O_CH * oc, O_CH * (oc + 1))
                    o_ps = pp.tile([BDK, O_CH], f32, name="o_ps", tag="big",
                                   bufs=4)
                    nc.tensor.matmul(o_ps[:], b_sb[:], xt_[:, sl],
                                     start=True, stop=True)
                    if state["nrelu"] % 2 == 0:
                        nc.scalar.activation(osb[:, sl], o_ps[:], Relu)
                    else:
                        nc.vector.tensor_scalar_max(osb[:, sl], o_ps[:], 0.0)
                    state["nrelu"] += 1
                queues[s % 3].dma_start(
                    y_d.ap()[:, s * HW:(s + 1) * HW], osb[:])

            # --- schedule: A(h0) | C0 | A(h1) interleaved with D(h0) | C1 |
            # D(h1) — half-1 loads/grams overlap half-0 outputs/stores ---
            for s in range(S0):
                emit_A_stack(s)
            drain_pend()
            emit_phase_C(range(T0))
            emit_attn_chain(0)
            for i in range(max(S0, S1)):
                if i < S1:
                    emit_A_stack(S0 + i)
                if i < S0:
                    emit_D_stack(i)
            drain_pend()
            emit_phase_C(range(T0, T))
            emit_attn_chain(S0)
            for s in range(S0, S):
                emit_D_stack(s)

    nc.compile()
    return nc


def _get_compiled(S0, S1, T0, T, have_bias: bool, used: tuple):
    key = (S0, S1, T0, T, have_bias, used)
    if key not in _cache:
        _cache[key] = _build(S0, S1, T0, T, have_bias, used)
    return _cache[key]


def _bd7(m: np.ndarray) -> np.ndarray:
    out = np.zeros((BDK, BDK), dtype=np.float32)
    for j in range(BD):
        out[K * j:K * (j + 1), K * j:K * (j + 1)] = m
    return out


def _plan(ids: np.ndarray):
    """Split persons into N_CORES contiguous chunks at imgid boundaries, and
    each core's chunk into two softmax-independent halves (again at a group
    boundary). Returns uniform per-core stack/tile counts."""
    change = np.flatnonzero(np.diff(ids)) + 1
    allb = np.concatenate([[0], change, [P_TOTAL]]).astype(np.int64)
    bounds = [0]
    for ci in range(1, N_CORES):
        target = P_TOTAL * ci / N_CORES
        cand = allb[allb > bounds[-1]]
        if len(cand) == 0:
            bounds.append(bounds[-1])
        else:
            bounds.append(int(cand[np.argmin(np.abs(cand - target))]))
    bounds.append(P_TOTAL)

    mids = []
    S0 = S1 = 1
    g0_max = g1_max = 1
    for ci in range(N_CORES):
        a, b = bounds[ci], bounds[ci + 1]
        pc = b - a
        if pc == 0:
            mids.append(0)
            continue
        lb = allb[(allb >= a) & (allb <= b)] - a  # group boundaries within
        m = int(lb[np.argmin(np.abs(lb - pc / 2))])
        mids.append(m)
        S0 = max(S0, math.ceil(m / BD))
        S1 = max(S1, math.ceil((pc - m) / BD))
        g0_max = max(g0_max, len(np.unique(ids[a:a + m])))
        g1_max = max(g1_max, len(np.unique(ids[a + m:b])))
    T0 = math.ceil((g0_max + 1) / BD)
    T1 = math.ceil((g1_max + 1) / BD)
    return bounds, mids, S0, S1, T0, T0 + T1


def _prepare(inputs: dict):
    import ml_dtypes
    nbf16 = ml_dtypes.bfloat16

    x = np.asarray(inputs["kpt_feat"], dtype=np.float32).reshape(
        P_TOTAL, K, HW)
    ids = np.asarray(inputs["imgid"]).astype(np.int64)
    Wq = np.asarray(inputs["Wq"], np.float32)
    Wk = np.asarray(inputs["Wk"], np.float32)
    Wv = np.asarray(inputs["Wv"], np.float32)
    bq = np.asarray(inputs["bq"], np.float32)
    bk = np.asarray(inputs["bk"], np.float32)
    bv = np.asarray(inputs["bv"], np.float32)

    bounds, mids, S0, S1, T0, T = _plan(ids)
    S = S0 + S1
    P_pad = S * BD

    wq_col = np.zeros((BDK, K), np.float32)
    for j in range(BD):
        wq_col[K * j:K * (j + 1), :] = Wq.T / NORM
    wkt_bd = _bd7(Wk.T.astype(np.float32))
    wv_aug = np.zeros((BDK, BDK + 1), np.float32)
    wv_aug[:, :BDK] = _bd7(Wv)
    for j in range(BD):
        wv_aug[K * j:K * (j + 1), BDK] = bv
    wv_aug = wv_aug.astype(nbf16)
    id119 = np.eye(BDK, dtype=np.float32).astype(nbf16)
    iaug = np.zeros((BDK + 1, BDK), np.float32)
    iaug[:BDK, :BDK] = np.eye(BDK, dtype=np.float32)
    bdmask = _bd7(np.ones((K, K), np.float32))

    have_bias = bool(np.any(bq) or np.any(bk))
    if have_bias:
        xsum = x.sum(axis=2)
        qx = xsum @ Wq.T
        kx = xsum @ Wk.T
        corr_all = (bk[None, :, None] * qx[:, None, :]
                    + bq[None, None, :] * kx[:, :, None]
                    + HW * (bq[None, None, :] * bk[None, :, None])) / NORM
        corr_all = corr_all.astype(np.float32)  # [P, m, i]
    else:
        corr_all = None

    xb = x.astype(nbf16)

    # person permutation (two padded halves) + selector tensors per core
    eye = np.eye(K, dtype=np.float32)
    sels = []
    newpos_all = []
    used_set = set()
    for ci in range(N_CORES):
        a, b = bounds[ci], bounds[ci + 1]
        pc = b - a
        m = mids[ci]
        # newpos[old person idx within core] -> padded position
        newpos = np.concatenate([np.arange(m),
                                 BD * S0 + np.arange(pc - m)])
        newpos_all.append(newpos)
        # slots per padded position: half0 groups at 0.., half1 at 7*T0..;
        # padding persons go to the halves' dummy slots
        slots = np.full((P_pad,), 7 * T - 1, np.int64)
        slots[:BD * S0] = 7 * T0 - 1
        if m:
            _, lg = np.unique(ids[a:a + m], return_inverse=True)
            slots[:m] = lg
        if pc - m:
            _, lg = np.unique(ids[a + m:b], return_inverse=True)
            slots[BD * S0:BD * S0 + (pc - m)] = 7 * T0 + lg
        sel = np.zeros((S, T, BDK, BDK), np.float32)
        for s in range(S):
            for j in range(BD):
                g = slots[BD * s + j]
                t, lgi = divmod(g, BD)
                sel[s, t, K * j:K * (j + 1), K * lgi:K * (lgi + 1)] = eye
                used_set.add((s, t))
        sels.append(sel)
    used = tuple(sorted(used_set))

    in_maps = []
    for ci in range(N_CORES):
        a, b = bounds[ci], bounds[ci + 1]
        pc = b - a
        # partition-major x: [120, S*HW]; row 119 = ones (residual fold)
        np_ = newpos_all[ci]
        rows = np.zeros((P_pad, K, HW), dtype=nbf16)
        if pc:
            rows[np_] = xb[a:b]
        rows = rows.reshape(P_pad * K, HW)
        arr3 = np.zeros((S, BDK + 1, HW), dtype=nbf16)
        arr3[:, :BDK] = rows.reshape(S, BDK, HW)
        arr3[:, BDK] = 1.0
        xs = np.ascontiguousarray(
            arr3.transpose(1, 0, 2).reshape(BDK + 1, S * HW))
        sel = sels[ci]
        su = np.stack([sel[s, t] for (s, t) in used])  # [U, 119, 119]
        sel_pack = su.transpose(1, 0, 2).reshape(BDK, len(used) * BDK)
        selt_pack = su.transpose(2, 0, 1).reshape(BDK, len(used) * BDK)
        m = {
            "x": xs,
            "wq_col": wq_col,
            "wkt_bd": wkt_bd,
            "wv_aug": wv_aug,
            "id119": id119,
            "iaug": iaug,
            "bdmask": bdmask,
            "bdmaskb": bdmask.astype(nbf16),
            "sel": np.ascontiguousarray(sel_pack).astype(nbf16),
            "selT": np.ascontiguousarray(selt_pack).astype(nbf16),
        }
        if have_bias:
            corr_col = np.zeros((BDK, K * S), np.float32)
            if pc:
                cpad = np.zeros((P_pad, K, K), np.float32)
                cpad[np_] = corr_all[a:b]
                for s in range(S):
                    for j in range(BD):
                        corr_col[K * j:K * (j + 1), K * s:K * (s + 1)] = \
                            cpad[BD * s + j]
            m["corr_col"] = corr_col
        in_maps.append(m)
    return in_maps, bounds, newpos_all, (S0, S1, T0, T), have_bias, used


def _gather(results, bounds, newpos_all, S):
    out = np.empty((P_TOTAL, K, 64, 64), dtype=np.float32)
    for ci in range(N_CORES):
        a, b = bounds[ci], bounds[ci + 1]
        pc = b - a
        if pc:
            y = np.asarray(results[ci]["y"], dtype=np.float32)  # [119, S*HW]
            y = y.reshape(BDK, S, HW).transpose(1, 0, 2).reshape(
                S * BD, K, 64, 64)
            out[a:b] = y[newpos_all[ci]]
    return out


def _run(inputs: dict, trace: bool = False):
    _ensure_path()
    from concourse.bass_utils import run_bass_kernel_spmd

    in_maps, bounds, newpos_all, (S0, S1, T0, T), have_bias, used = \
        _prepare(inputs)
    nc = _get_compiled(S0, S1, T0, T, have_bias, used)
    res = run_bass_kernel_spmd(nc, in_maps, list(range(N_CORES)), trace=trace)
    return _gather(res.results, bounds, newpos_all, S0 + S1), res


def kernel(**inputs) -> np.ndarray:
    out, _ = _run(inputs, trace=False)
    return out



# revision 3
# speedup vs baseline: 1.1781x; 1.1781x over previous
"""Trainium2 Bass kernel for nn_JointRelationModule (self-contained).

Math (per person p; softmax is segment-softmax over persons within an imgid
group, elementwise over the (K,K) score entries):
    q = Wq x + bq ; k = Wk x + bk ; v = Wv x + bv      (1x1 conv over K=17)
    S_p = q_p k_p^T / 64
    attn = segment-softmax over persons
    out = relu(attn_p @ v_p + x_p)

Device formulation (heavy ops bf16 on the PE, block-column layouts):
  - Stack BD=7 persons as [119, hw]. Per stack: G = x x^T via PE transpose +
    accumulating matmuls (bf16, f32 PSUM).
  - scores^T in block-column layout [119, 17] via a masked-Gram matmul chain
    (block-diag mask kills cross-person terms), so no gather/scatter DMAs.
  - Segment softmax via per-stack selector matmuls into group-slot tiles,
    reciprocal, selector-transpose broadcast back; all partition-aligned.
  - Output: B = blockdiag((attn Wv)^T) + I with an av row appended; the
    residual and v-bias ride along x_aug (all-ones row), so each output chunk
    is one matmul + one relu. B is zero-padded to 128 weight columns so the
    PE fast-weight-load path kicks in. Stored bf16, host upcasts.

Data movement: x and y live in a partition-major layout [120, S*hw]. All bulk
x loads / y stores ride the gpsimd (SWDGE) ring: its descriptors spread
evenly over all 16 SDMA engines, unlike the HWDGE (sync/scalar) rings which
concentrate on engines 0-6. Small constants ride the otherwise-idle HWDGE
rings. Stack 0's load is column-chunked so the first transposes start early.

Sharding: data-parallel over persons at imgid group boundaries (8 cores),
weights replicated. Host casts x to bf16 (halves load bytes); output comes
back bf16 (halves store bytes). Tolerance 2e-2; measured error ~5e-3.
"""

import math
import sys

import numpy as np

K = 17
HW = 4096  # 64*64
P_TOTAL = 512
N_CORES = 8
NORM = 64.0
BD = 7          # persons per stack
BDK = BD * K    # 119
O_CH = 512      # output chunk cols (one PSUM bank of f32)
WPAD = 128      # output-matmul weight cols padded for fast weight load

_cache: dict = {}


def _ensure_path():
    try:
        import concourse.bass  # noqa: F401
    except ImportError:
        for p in ("/opt/trn_rl_repo", "/root/.axon_site/_ro/trn_rl_repo"):
            if p not in sys.path:
                sys.path.insert(0, p)
        import concourse.bass  # noqa: F401


def _build(S0: int, S1: int, T0: int, T: int, have_bias: bool, used: tuple):
    """Builds + compiles the per-core SPMD Bass program.

    Persons are split into two independent softmax halves (stacks [0,S0) with
    group-slot tiles [0,T0), stacks [S0,S) with tiles [T0,T)) so half-1's
    loads/grams overlap half-0's outputs/stores."""
    _ensure_path()
    import concourse.bacc as bacc
    import concourse.mybir as mybir
    import concourse.tile as tile

    f32 = mybir.dt.float32
    bf16 = mybir.dt.bfloat16
    Exp = mybir.ActivationFunctionType.Exp
    Relu = mybir.ActivationFunctionType.Relu

    S = S0 + S1
    U = len(used)

    nc = bacc.Bacc(
        "TRN2",
        target_bir_lowering=False,
        debug=False,
        enable_asserts=False,
        num_devices=N_CORES,
    )

    x_d = nc.dram_tensor("x", [BDK + 1, S * HW], bf16, kind="ExternalInput")
    wq_d = nc.dram_tensor("wq_col", [BDK, K], f32, kind="ExternalInput")
    wk_d = nc.dram_tensor("wkt_bd", [BDK, BDK], f32, kind="ExternalInput")
    wv_d = nc.dram_tensor("wv_aug", [BDK, BDK + 1], bf16, kind="ExternalInput")
    id_d = nc.dram_tensor("id119", [BDK, BDK], bf16, kind="ExternalInput")
    ia_d = nc.dram_tensor("iaug", [BDK + 1, BDK], f32, kind="ExternalInput")
    mk_d = nc.dram_tensor("bdmask", [BDK, BDK], f32, kind="ExternalInput")
    mkb_d = nc.dram_tensor("bdmaskb", [BDK, BDK], bf16, kind="ExternalInput")
    sel_d = nc.dram_tensor("sel", [BDK, U * BDK], bf16, kind="ExternalInput")
    selt_d = nc.dram_tensor("selT", [BDK, U * BDK], bf16,
                            kind="ExternalInput")
    if have_bias:
        corr_d = nc.dram_tensor("corr_col", [BDK, K * S], f32,
                                kind="ExternalInput")
    y_d = nc.dram_tensor("y", [BDK, S * HW], bf16, kind="ExternalOutput")

    G_CH = 1024          # x cols per transpose group
    n_grp = HW // G_CH   # 4 groups per stack

    with tile.TileContext(nc) as tc:
        with (
            nc.allow_low_precision(reason="bf16 softmax ok at 2e-2 tol"),
            tc.tile_pool(name="xpool", bufs=1) as xpool,
            tc.tile_pool(name="cpool", bufs=1) as cpool,
            tc.tile_pool(name="wpool", bufs=2) as wpool,
            tc.tile_pool(name="opool", bufs=2) as opool,
            tc.tile_pool(name="pp", bufs=2, space="PSUM") as pp,
        ):
            # --- tiny phase-A constants first (id_t gates every transpose);
            # consts ride the HWDGE rings, bulk x/y rides the SWDGE ring ---
            id_t = cpool.tile([BDK, BDK], bf16, name="id_t", tag="id")
            mk_t = cpool.tile([BDK, BDK], f32, name="mk_t", tag="mk")
            wq_t = cpool.tile([BDK, K], f32, name="wq_t", tag="wq")
            wk_t = cpool.tile([BDK, BDK], f32, name="wk_t", tag="wk")
            nc.sync.dma_start(id_t[:], id_d.ap())
            nc.scalar.dma_start(mk_t[:], mk_d.ap())
            nc.sync.dma_start(wq_t[:], wq_d.ap())
            nc.scalar.dma_start(wk_t[:], wk_d.ap())

            x_tiles = []  # per stack

            def load_xtile(s):
                xt_ = xpool.tile([BDK + 1, HW], bf16, name=f"xp{s}",
                                 tag=f"xp{s}")
                base = s * HW
                if s == 0:
                    # chunked so the first transposes start ASAP
                    for gi in range(n_grp):
                        csl = slice(base + G_CH * gi, base + G_CH * (gi + 1))
                        nc.gpsimd.dma_start(
                            xt_[:, G_CH * gi:G_CH * (gi + 1)],
                            x_d.ap()[:, csl])
                else:
                    nc.gpsimd.dma_start(xt_[:], x_d.ap()[:, base:base + HW])
                x_tiles.append(xt_)

            load_xtile(0)
            if have_bias:
                corr_t = cpool.tile([BDK, K * S], f32, name="corr_t",
                                    tag="corr")
                nc.scalar.dma_start(corr_t[:], corr_d.ap())

            # bulkier constants: tiles declared now, DMAs emitted mid-phase-A
            # (the framework coalesces DMA waits into a cumulative counter, so
            # anything emitted before the first transpose delays it)
            wv_t = cpool.tile([BDK, BDK + 1], bf16, name="wv_t", tag="wv")
            ia_t = cpool.tile([BDK + 1, BDK], f32, name="ia_t", tag="ia")
            sel_t = cpool.tile([BDK, U * BDK], bf16, name="sel_t", tag="sel")
            selt_t = cpool.tile([BDK, U * BDK], bf16, name="selt_t",
                                tag="selt")
            mkb_t = cpool.tile([BDK, BDK], bf16, name="mkb_t", tag="mkb")

            def emit_const_dmas():
                nc.sync.dma_start(wv_t[:], wv_d.ap())
                nc.scalar.dma_start(ia_t[:], ia_d.ap())
                nc.sync.dma_start(mkb_t[:], mkb_d.ap())
                nc.scalar.dma_start(sel_t[:], sel_d.ap())
                nc.sync.dma_start(selt_t[:], selt_d.ap())

            exp_all = cpool.tile([BDK, K * S], bf16, name="exp_all", tag="exp")
            inv_t = cpool.tile([BDK, K * T], bf16, name="inv_t", tag="inv")

            # --- phase A: transpose -> gram -> scores^T -> exp, skewed ---
            # PSUM tags (8 banks): big=tp/o_ps x4, gsb=g/seg/b x2, tiny x2
            TC = BDK + 1         # 120: chunk col stride (4B-aligned in PSUM)
            state = {"ncopy": 0, "loaded": 1}
            g_tiles = {}

            def emit_transposes(s, gi):
                xt_ = x_tiles[s]
                tp = pp.tile([128, 8 * TC], bf16, name="tp", tag="big",
                             bufs=4)
                for c8 in range(8):
                    col = G_CH * gi + 128 * c8
                    nc.tensor.transpose(
                        tp[:, TC * c8:TC * c8 + BDK],
                        xt_[0:BDK, col:col + 128], id_t[:],
                    )
                xt = wpool.tile([128, 8 * TC], bf16, name="xt", tag="xt",
                                bufs=4)
                if state["ncopy"] % 2 == 0:
                    nc.vector.tensor_copy(xt[:], tp[:])
                else:
                    nc.scalar.copy(xt[:], tp[:])
                state["ncopy"] += 1
                return xt

            def emit_gram(s, gi, xt):
                if s not in g_tiles:
                    g_tiles[s] = pp.tile([BDK + 1, BDK], f32, name=f"g{s}",
                                         tag="gsb", bufs=2)
                g_ps = g_tiles[s]
                for c8 in range(8):
                    nc.tensor.matmul(
                        g_ps[0:BDK, :], xt[:, TC * c8:TC * c8 + BDK],
                        xt[:, TC * c8:TC * c8 + BDK],
                        start=(gi == 0 and c8 == 0),
                        stop=(gi == n_grp - 1 and c8 == 7),
                    )
                if gi == n_grp - 1:
                    emit_tiny_chain(s)

            def emit_tiny_chain(s):
                g_sb = wpool.tile([BDK, BDK], f32, name="g_sb", tag="g_sb",
                                  bufs=2)
                nc.vector.tensor_mul(g_sb[:], g_tiles[s][0:BDK, :], mk_t[:])
                m1_ps = pp.tile([BDK, K], f32, name="m1", tag="tiny", bufs=2)
                nc.tensor.matmul(m1_ps[:], g_sb[:], wq_t[:], start=True,
                                 stop=True)
                m1_sb = wpool.tile([BDK, K], f32, name="m1_sb", tag="m1_sb",
                                   bufs=2)
                nc.scalar.copy(m1_sb[:], m1_ps[:])
                st_ps = pp.tile([BDK, K], f32, name="st", tag="tiny", bufs=2)
                nc.tensor.matmul(st_ps[:], wk_t[:], m1_sb[:], start=True,
                                 stop=True)
                esl = slice(K * s, K * (s + 1))
                if have_bias:
                    eb_sb = wpool.tile([BDK, K], f32, name="eb_sb", tag="eb")
                    nc.vector.tensor_add(eb_sb[:], st_ps[:], corr_t[:, esl])
                    nc.scalar.activation(exp_all[:, esl], eb_sb[:], Exp)
                else:
                    nc.scalar.activation(exp_all[:, esl], st_ps[:], Exp)

            pend = []

            def emit_A_stack(s):
                if s == 2:
                    emit_const_dmas()
                for gi in range(n_grp):
                    pend.append((s, gi, emit_transposes(s, gi)))
                    # just-in-time prefetch, interleaved with compute emission
                    while state["loaded"] < min(s + 3, S):
                        load_xtile(state["loaded"])
                        state["loaded"] += 1
                    if len(pend) > 2:
                        ps, pgi, xt = pend.pop(0)
                        emit_gram(ps, pgi, xt)

            def drain_pend():
                while pend:
                    ps, pgi, xt = pend.pop(0)
                    emit_gram(ps, pgi, xt)

            # --- phase C (per half): segment sums -> reciprocal ---
            def emit_phase_C(trange):
                seg_tiles = {}
                for t in trange:
                    idxs = [i for i, (ss, tt) in enumerate(used) if tt == t]
                    seg_ps = pp.tile([BDK + 1, BDK], f32, name=f"seg{t}",
                                     tag="gsb", bufs=2)
                    for n, i in enumerate(idxs):
                        s = used[i][0]
                        nc.tensor.matmul(
                            seg_ps[0:BDK, 0:K],
                            sel_t[:, BDK * i:BDK * (i + 1)],
                            exp_all[:, K * s:K * (s + 1)],
                            start=(n == 0), stop=(n == len(idxs) - 1),
                        )
                    seg_tiles[t] = seg_ps
                for t in trange:
                    seg_sb = wpool.tile([BDK, K], f32, name="seg_sb",
                                        tag="seg_sb")
                    nc.vector.tensor_scalar_max(
                        seg_sb[:], seg_tiles[t][0:BDK, 0:K], 1e-30)
                    nc.vector.reciprocal(inv_t[:, K * t:K * (t + 1)],
                                         seg_sb[:])

            # --- phase D: pipelined per stack ---
            state["nrelu"] = 0
            attn_tiles = {}

            def emit_attn_chain(s):
                idxs = [i for i, (ss, tt) in enumerate(used) if ss == s]
                invb_ps = pp.tile([BDK, K], f32, name="invb", tag="tiny",
                                  bufs=2)
                for n, i in enumerate(idxs):
                    t = used[i][1]
                    nc.tensor.matmul(
                        invb_ps[:],
                        selt_t[:, BDK * i:BDK * (i + 1)],
                        inv_t[:, K * t:K * (t + 1)],
                        start=(n == 0), stop=(n == len(idxs) - 1),
                    )
                attn_sb = wpool.tile([BDK, K], bf16, name="attn_sb",
                                     tag="attn_c", bufs=2)
                nc.vector.tensor_mul(attn_sb[:], exp_all[:, K * s:K * (s + 1)],
                                     invb_ps[:])
                # block-diag mask applied in one op via a 0-stride broadcast
                attn_bd = wpool.tile([BDK, BDK], bf16, name="attn_bd",
                                     tag="attn", bufs=2)
                attn_rep = attn_sb[:].unsqueeze(1).to_broadcast((BDK, BD, K))
                nc.vector.tensor_mul(attn_bd[:], attn_rep, mkb_t[:])
                attn_tiles[s] = attn_bd

            def emit_D_stack(s):
                b_ps = pp.tile([BDK + 1, BDK], f32, name="b_ps", tag="gsb",
                               bufs=2)
                nc.tensor.matmul(b_ps[:], wv_t[:], attn_tiles.pop(s)[:],
                                 start=True, stop=True)
                # pad B to 128 weight cols: enables PE fast weight load on
                # the output matmuls (extra PSUM rows are never read)
                b_sb = wpool.tile([BDK + 1, WPAD], bf16, name="b_sb", tag="B",
                                  bufs=2)
                nc.vector.tensor_add(b_sb[:, 0:BDK], b_ps[:], ia_t[:])
                nc.gpsimd.memset(b_sb[:, BDK:WPAD], 0.0)
                if s + 1 < S and s + 1 != S0:
                    emit_attn_chain(s + 1)  # overlaps this stack's matmuls

                osb = opool.tile([BDK, HW], bf16, name="osb", tag="osb",
                                 bufs=3)
                xt_ = x_tiles[s]
                for oc in range(HW // O_CH):
                    sl = slice(O_CH * oc, O_CH * (oc + 1))
                    o_ps = pp.tile([WPAD, O_CH], f32, name="o_ps", tag="big",
                                   bufs=4)
                    nc.tensor.matmul(o_ps[:], b_sb[:], xt_[:, sl],
                                     start=True, stop=True)
                    if state["nrelu"] % 2 == 0:
                        nc.scalar.activation(osb[:, sl], o_ps[0:BDK, :], Relu)
                    else:
                        nc.vector.tensor_scalar_max(osb[:, sl],
                                                    o_ps[0:BDK, :], 0.0)
                    state["nrelu"] += 1
                nc.gpsimd.dma_start(
                    y_d.ap()[:, s * HW:(s + 1) * HW], osb[:])

            # --- schedule: A(h0) | C0 | A(h1) interleaved with D(h0) | C1 |
            # D(h1) — half-1 loads/grams overlap half-0 outputs/stores ---
            for s in range(S0):
                emit_A_stack(s)
            drain_pend()
            emit_phase_C(range(T0))
            emit_attn_chain(0)
            for i in range(max(S0, S1)):
                if i < S1:
                    emit_A_stack(S0 + i)
                if i < S0:
                    emit_D_stack(i)
            drain_pend()
            emit_phase_C(range(T0, T))
            emit_attn_chain(S0)
            for s in range(S0, S):
                emit_D_stack(s)

    nc.compile()
    return nc


def _get_compiled(S0, S1, T0, T, have_bias: bool, used: tuple):
    key = (S0, S1, T0, T, have_bias, used)
    if key not in _cache:
        _cache[key] = _build(S0, S1, T0, T, have_bias, used)
    return _cache[key]


def _bd7(m: np.ndarray) -> np.ndarray:
    out = np.zeros((BDK, BDK), dtype=np.float32)
    for j in range(BD):
        out[K * j:K * (j + 1), K * j:K * (j + 1)] = m
    return out


def _plan(ids: np.ndarray):
    """Split persons into N_CORES contiguous chunks at imgid boundaries, each
    chunk into two softmax-independent halves (again at group boundaries).
    All cores run the same compiled program padded to (S0, S1) stacks, so the
    only objective is minimizing S0+S1: greedy furthest-reach per core."""
    change = np.flatnonzero(np.diff(ids)) + 1
    allb = np.concatenate([[0], change, [P_TOTAL]]).astype(np.int64)

    def plan_for(S0, S1):
        bounds, mids = [0], []
        for _ in range(N_CORES):
            a = bounds[-1]
            # furthest b reachable: some boundary m with m-a <= 7*S0 and
            # b-m <= 7*S1
            ms = allb[(allb >= a) & (allb <= a + BD * S0)]
            best_b, best_m = a, a
            for m in ms:
                cand = allb[(allb >= m) & (allb <= m + BD * S1)]
                if len(cand) and cand[-1] > best_b:
                    best_b, best_m = int(cand[-1]), int(m)
            bounds.append(best_b)
            mids.append(best_m - a)
            if best_b == P_TOTAL:
                break
        while len(bounds) < N_CORES + 1:
            bounds.append(bounds[-1])
            mids.append(0)
        return (bounds, mids) if bounds[-1] == P_TOTAL else None

    Smin = math.ceil(P_TOTAL / (N_CORES * BD))
    best = None
    for Stot in range(Smin, Smin + 4):
        opts = sorted(range(1, Stot), key=lambda s0: abs(s0 - Stot / 2))
        for S0 in opts:
            r = plan_for(S0, Stot - S0)
            if r is not None:
                best = (S0, Stot - S0) + r
                break
        if best:
            break
    assert best is not None
    S0, S1, bounds, mids = best

    g0_max = g1_max = 1
    for ci in range(N_CORES):
        a, b = bounds[ci], bounds[ci + 1]
        m = mids[ci]
        if m:
            g0_max = max(g0_max, len(np.unique(ids[a:a + m])))
        if b - (a + m):
            g1_max = max(g1_max, len(np.unique(ids[a + m:b])))
    T0 = math.ceil((g0_max + 1) / BD)
    T1 = math.ceil((g1_max + 1) / BD)
    return bounds, mids, S0, S1, T0, T0 + T1


def _prepare(inputs: dict):
    import ml_dtypes
    nbf16 = ml_dtypes.bfloat16

    x = np.asarray(inputs["kpt_feat"], dtype=np.float32).reshape(
        P_TOTAL, K, HW)
    ids = np.asarray(inputs["imgid"]).astype(np.int64)
    Wq = np.asarray(inputs["Wq"], np.float32)
    Wk = np.asarray(inputs["Wk"], np.float32)
    Wv = np.asarray(inputs["Wv"], np.float32)
    bq = np.asarray(inputs["bq"], np.float32)
    bk = np.asarray(inputs["bk"], np.float32)
    bv = np.asarray(inputs["bv"], np.float32)

    bounds, mids, S0, S1, T0, T = _plan(ids)
    S = S0 + S1
    P_pad = S * BD

    wq_col = np.zeros((BDK, K), np.float32)
    for j in range(BD):
        wq_col[K * j:K * (j + 1), :] = Wq.T / NORM
    wkt_bd = _bd7(Wk.T.astype(np.float32))
    wv_aug = np.zeros((BDK, BDK + 1), np.float32)
    wv_aug[:, :BDK] = _bd7(Wv)
    for j in range(BD):
        wv_aug[K * j:K * (j + 1), BDK] = bv
    wv_aug = wv_aug.astype(nbf16)
    id119 = np.eye(BDK, dtype=np.float32).astype(nbf16)
    iaug = np.zeros((BDK + 1, BDK), np.float32)
    iaug[:BDK, :BDK] = np.eye(BDK, dtype=np.float32)
    bdmask = _bd7(np.ones((K, K), np.float32))

    have_bias = bool(np.any(bq) or np.any(bk))
    if have_bias:
        xsum = x.sum(axis=2)
        qx = xsum @ Wq.T
        kx = xsum @ Wk.T
        corr_all = (bk[None, :, None] * qx[:, None, :]
                    + bq[None, None, :] * kx[:, :, None]
                    + HW * (bq[None, None, :] * bk[None, :, None])) / NORM
        corr_all = corr_all.astype(np.float32)  # [P, m, i]
    else:
        corr_all = None

    xb = x.astype(nbf16)

    # person permutation (two padded halves) + selector tensors per core
    eye = np.eye(K, dtype=np.float32)
    sels = []
    newpos_all = []
    used_set = set()
    for ci in range(N_CORES):
        a, b = bounds[ci], bounds[ci + 1]
        pc = b - a
        m = mids[ci]
        # newpos[old person idx within core] -> padded position
        newpos = np.concatenate([np.arange(m),
                                 BD * S0 + np.arange(pc - m)])
        newpos_all.append(newpos)
        # slots per padded position: half0 groups at 0.., half1 at 7*T0..;
        # padding persons go to the halves' dummy slots
        slots = np.full((P_pad,), 7 * T - 1, np.int64)
        slots[:BD * S0] = 7 * T0 - 1
        if m:
            _, lg = np.unique(ids[a:a + m], return_inverse=True)
            slots[:m] = lg
        if pc - m:
            _, lg = np.unique(ids[a + m:b], return_inverse=True)
            slots[BD * S0:BD * S0 + (pc - m)] = 7 * T0 + lg
        sel = np.zeros((S, T, BDK, BDK), np.float32)
        for s in range(S):
            for j in range(BD):
                g = slots[BD * s + j]
                t, lgi = divmod(g, BD)
                sel[s, t, K * j:K * (j + 1), K * lgi:K * (lgi + 1)] = eye
                used_set.add((s, t))
        sels.append(sel)
    used = tuple(sorted(used_set))

    in_maps = []
    for ci in range(N_CORES):
        a, b = bounds[ci], bounds[ci + 1]
        pc = b - a
        # partition-major x: [120, S*HW]; row 119 = ones (residual fold)
        np_ = newpos_all[ci]
        rows = np.zeros((P_pad, K, HW), dtype=nbf16)
        if pc:
            rows[np_] = xb[a:b]
        rows = rows.reshape(P_pad * K, HW)
        arr3 = np.zeros((S, BDK + 1, HW), dtype=nbf16)
        arr3[:, :BDK] = rows.reshape(S, BDK, HW)
        arr3[:, BDK] = 1.0
        xs = np.ascontiguousarray(
            arr3.transpose(1, 0, 2).reshape(BDK + 1, S * HW))
        sel = sels[ci]
        su = np.stack([sel[s, t] for (s, t) in used])  # [U, 119, 119]
        sel_pack = su.transpose(1, 0, 2).reshape(BDK, len(used) * BDK)
        selt_pack = su.transpose(2, 0, 1).reshape(BDK, len(used) * BDK)
        m = {
            "x": xs,
            "wq_col": wq_col,
            "wkt_bd": wkt_bd,
            "wv_aug": wv_aug,
            "id119": id119,
            "iaug": iaug,
            "bdmask": bdmask,
            "bdmaskb": bdmask.astype(nbf16),
            "sel": np.ascontiguousarray(sel_pack).astype(nbf16),
            "selT": np.ascontiguousarray(selt_pack).astype(nbf16),
        }
        if have_bias:
            corr_col = np.zeros((BDK, K * S), np.float32)
            if pc:
                cpad = np.zeros((P_pad, K, K), np.float32)
                cpad[np_] = corr_all[a:b]
                for s in range(S):
                    for j in range(BD):
                        corr_col[K * j:K * (j + 1), K * s:K * (s + 1)] = \
                            cpad[BD * s + j]
            m["corr_col"] = corr_col
        in_maps.append(m)
    return in_maps, bounds, newpos_all, (S0, S1, T0, T), have_bias, used


def _gather(results, bounds, newpos_all, S):
    out = np.empty((P_TOTAL, K, 64, 64), dtype=np.float32)
    for ci in range(N_CORES):
        a, b = bounds[ci], bounds[ci + 1]
        pc = b - a
        if pc:
            y = np.asarray(results[ci]["y"], dtype=np.float32)  # [119, S*HW]
            y = y.reshape(BDK, S, HW).transpose(1, 0, 2).reshape(
                S * BD, K, 64, 64)
            out[a:b] = y[newpos_all[ci]]
    return out


def _run(inputs: dict, trace: bool = False):
    _ensure_path()
    from concourse.bass_utils import run_bass_kernel_spmd

    in_maps, bounds, newpos_all, (S0, S1, T0, T), have_bias, used = \
        _prepare(inputs)
    nc = _get_compiled(S0, S1, T0, T, have_bias, used)
    res = run_bass_kernel_spmd(nc, in_maps, list(range(N_CORES)), trace=trace)
    return _gather(res.results, bounds, newpos_all, S0 + S1), res


def kernel(**inputs) -> np.ndarray:
    out, _ = _run(inputs, trace=False)
    return out


# revision 16
# speedup vs baseline: 1.2661x; 1.0747x over previous
"""Trainium2 Bass kernel for nn_JointRelationModule (self-contained).

Math (per person p; softmax is segment-softmax over persons within an imgid
group, elementwise over the (K,K) score entries):
    q = Wq x + bq ; k = Wk x + bk ; v = Wv x + bv      (1x1 conv over K=17)
    S_p = q_p k_p^T / 64
    attn = segment-softmax over persons
    out = relu(attn_p @ v_p + x_p)

Device formulation (heavy ops bf16 on the PE, block-column layouts):
  - Stack BD=7 persons as [119, hw]. Per stack: G = x x^T via PE transpose +
    accumulating matmuls (bf16, f32 PSUM).
  - scores^T in block-column layout [119, 17] via a masked-Gram matmul chain
    (block-diag mask kills cross-person terms), so no gather/scatter DMAs.
  - Segment softmax via per-stack selector matmuls into group-slot tiles,
    reciprocal, selector-transpose broadcast back; all partition-aligned.
  - Output: B = blockdiag((attn Wv)^T) + I with an av row appended; the
    residual and v-bias ride along x_aug (all-ones row), so each output chunk
    is one matmul + one relu. B is zero-padded to 128 weight columns so the
    PE fast-weight-load path kicks in. Stored bf16, host upcasts.

Data movement: x and y live in a partition-major layout [120, S*hw]. All bulk
x loads / y stores ride the gpsimd (SWDGE) ring: its descriptors spread
evenly over all 16 SDMA engines, unlike the HWDGE (sync/scalar) rings which
concentrate on engines 0-6. Small constants ride the otherwise-idle HWDGE
rings. Stack 0's load is column-chunked so the first transposes start early.

Sharding: data-parallel over persons at imgid group boundaries (8 cores),
weights replicated. Host casts x to bf16 (halves load bytes); output comes
back bf16 (halves store bytes). Tolerance 2e-2; measured error ~5e-3.
"""

import math
import sys

import numpy as np

K = 17
HW = 4096  # 64*64
P_TOTAL = 512
N_CORES = 8
NORM = 64.0
BD = 7          # persons per stack
BDK = BD * K    # 119
O_CH = 512      # output chunk cols (one PSUM bank of f32)
WPAD = 128      # output-matmul weight cols padded for fast weight load

_cache: dict = {}


def _ensure_path():
    try:
        import concourse.bass  # noqa: F401
    except ImportError:
        for p in ("/opt/trn_rl_repo", "/root/.axon_site/_ro/trn_rl_repo"):
            if p not in sys.path:
                sys.path.insert(0, p)
        import concourse.bass  # noqa: F401


def _build(S_list: tuple, T_list: tuple, have_bias: bool, used: tuple):
    """Builds + compiles the per-core SPMD Bass program.

    Persons are split into NP softmax-independent phases (phase p owns stacks
    [cumS[p], cumS[p+1]) and group-slot tiles [cumT[p], cumT[p+1])); phase
    p+1's loads/grams interleave stack-wise with phase p's outputs/stores so
    DMA loads and stores flow continuously."""
    _ensure_path()
    import concourse.bacc as bacc
    import concourse.mybir as mybir
    import concourse.tile as tile

    f32 = mybir.dt.float32
    bf16 = mybir.dt.bfloat16
    Exp = mybir.ActivationFunctionType.Exp
    Relu = mybir.ActivationFunctionType.Relu

    NP = len(S_list)
    S = sum(S_list)
    T = sum(T_list)
    cumS = [0]
    for sp in S_list:
        cumS.append(cumS[-1] + sp)
    cumT = [0]
    for tp in T_list:
        cumT.append(cumT[-1] + tp)
    phase_start = set(cumS[:-1])
    U = len(used)

    nc = bacc.Bacc(
        "TRN2",
        target_bir_lowering=False,
        debug=False,
        enable_asserts=False,
        num_devices=N_CORES,
    )

    x_d = nc.dram_tensor("x", [BDK + 1, S * HW], bf16, kind="ExternalInput")
    wq_d = nc.dram_tensor("wq_col", [BDK, K], f32, kind="ExternalInput")
    wk_d = nc.dram_tensor("wkt_bd", [BDK, BDK], f32, kind="ExternalInput")
    wv_d = nc.dram_tensor("wv_aug", [BDK, BDK + 1], bf16, kind="ExternalInput")
    id_d = nc.dram_tensor("id119", [BDK, BDK], bf16, kind="ExternalInput")
    ia_d = nc.dram_tensor("iaug", [BDK + 1, BDK], f32, kind="ExternalInput")
    mk_d = nc.dram_tensor("bdmask", [BDK, BDK], f32, kind="ExternalInput")
    mkb_d = nc.dram_tensor("bdmaskb", [BDK, BDK], bf16, kind="ExternalInput")
    sel_d = nc.dram_tensor("sel", [BDK, U * BDK], bf16, kind="ExternalInput")
    selt_d = nc.dram_tensor("selT", [BDK, U * BDK], bf16,
                            kind="ExternalInput")
    if have_bias:
        corr_d = nc.dram_tensor("corr_col", [BDK, K * S], f32,
                                kind="ExternalInput")
    y_d = nc.dram_tensor("y", [BDK, S * HW], bf16, kind="ExternalOutput")

    G_CH = 1024          # x cols per transpose group
    n_grp = HW // G_CH   # 4 groups per stack

    with tile.TileContext(nc) as tc:
        with (
            nc.allow_low_precision(reason="bf16 softmax ok at 2e-2 tol"),
            tc.tile_pool(name="xpool", bufs=1) as xpool,
            tc.tile_pool(name="cpool", bufs=1) as cpool,
            tc.tile_pool(name="wpool", bufs=2) as wpool,
            tc.tile_pool(name="opool", bufs=2) as opool,
            tc.tile_pool(name="pp", bufs=2, space="PSUM") as pp,
        ):
            # --- tiny phase-A constants first (id_t gates every transpose);
            # consts ride the HWDGE rings, bulk x/y rides the SWDGE ring ---
            id_t = cpool.tile([BDK, BDK], bf16, name="id_t", tag="id")
            mk_t = cpool.tile([BDK, BDK], f32, name="mk_t", tag="mk")
            wq_t = cpool.tile([BDK, K], f32, name="wq_t", tag="wq")
            wk_t = cpool.tile([BDK, BDK], f32, name="wk_t", tag="wk")
            nc.sync.dma_start(id_t[:], id_d.ap())
            nc.scalar.dma_start(mk_t[:], mk_d.ap())
            nc.sync.dma_start(wq_t[:], wq_d.ap())
            nc.scalar.dma_start(wk_t[:], wk_d.ap())

            x_tiles = []  # per stack

            def load_xtile(s):
                xt_ = xpool.tile([BDK + 1, HW], bf16, name=f"xp{s}",
                                 tag=f"xp{s}")
                base = s * HW
                if s == 0:
                    # chunked so the first transposes start ASAP
                    for gi in range(n_grp):
                        csl = slice(base + G_CH * gi, base + G_CH * (gi + 1))
                        nc.gpsimd.dma_start(
                            xt_[:, G_CH * gi:G_CH * (gi + 1)],
                            x_d.ap()[:, csl])
                else:
                    nc.gpsimd.dma_start(xt_[:], x_d.ap()[:, base:base + HW])
                x_tiles.append(xt_)

            load_xtile(0)
            if have_bias:
                corr_t = cpool.tile([BDK, K * S], f32, name="corr_t",
                                    tag="corr")
                nc.scalar.dma_start(corr_t[:], corr_d.ap())

            # bulkier constants: tiles declared now, DMAs emitted mid-phase-A
            # (the framework coalesces DMA waits into a cumulative counter, so
            # anything emitted before the first transpose delays it)
            wv_t = cpool.tile([BDK, BDK + 1], bf16, name="wv_t", tag="wv")
            ia_t = cpool.tile([BDK + 1, BDK], f32, name="ia_t", tag="ia")
            sel_t = cpool.tile([BDK, U * BDK], bf16, name="sel_t", tag="sel")
            selt_t = cpool.tile([BDK, U * BDK], bf16, name="selt_t",
                                tag="selt")
            mkb_t = cpool.tile([BDK, BDK], bf16, name="mkb_t", tag="mkb")

            def emit_const_dmas():
                nc.sync.dma_start(wv_t[:], wv_d.ap())
                nc.scalar.dma_start(ia_t[:], ia_d.ap())
                nc.sync.dma_start(mkb_t[:], mkb_d.ap())
                nc.scalar.dma_start(sel_t[:], sel_d.ap())
                nc.sync.dma_start(selt_t[:], selt_d.ap())

            exp_all = cpool.tile([BDK, K * S], bf16, name="exp_all", tag="exp")
            inv_t = cpool.tile([BDK, K * T], bf16, name="inv_t", tag="inv")

            # --- phase A: transpose -> gram -> scores^T -> exp, skewed ---
            # PSUM tags (8 banks): big=tp/o_ps x4, gsb=g/seg/b x2, tiny x2
            TC = BDK + 1         # 120: chunk col stride (4B-aligned in PSUM)
            state = {"ncopy": 0, "loaded": 1, "a_left": S, "d_after_a": 0}
            g_tiles = {}

            # engine split: PSUM-reading elementwise work can only run on
            # DVE/ACT (GpSimd has no PSUM access). During A/D overlap, keep
            # A's copies on ACT and D's relus mostly on DVE so the in-order
            # engine queues don't cross-block; use both engines otherwise.
            def emit_transposes(s, gi):
                xt_ = x_tiles[s]
                tp = pp.tile([128, 8 * TC], bf16, name="tp", tag="big",
                             bufs=4)
                for c8 in range(8):
                    col = G_CH * gi + 128 * c8
                    nc.tensor.transpose(
                        tp[:, TC * c8:TC * c8 + BDK],
                        xt_[0:BDK, col:col + 128], id_t[:],
                    )
                xt = wpool.tile([128, 8 * TC], bf16, name="xt", tag="xt",
                                bufs=4)
                if state["interleave"] or state["ncopy"] % 2 == 1:
                    nc.scalar.copy(xt[:], tp[:])
                else:
                    nc.vector.tensor_copy(xt[:], tp[:])
                state["ncopy"] += 1
                return xt

            def emit_gram(s, gi, xt):
                if s not in g_tiles:
                    g_tiles[s] = pp.tile([BDK + 1, BDK], f32, name=f"g{s}",
                                         tag="gsb", bufs=2)
                g_ps = g_tiles[s]
                for c8 in range(8):
                    nc.tensor.matmul(
                        g_ps[0:BDK, :], xt[:, TC * c8:TC * c8 + BDK],
                        xt[:, TC * c8:TC * c8 + BDK],
                        start=(gi == 0 and c8 == 0),
                        stop=(gi == n_grp - 1 and c8 == 7),
                    )
                if gi == n_grp - 1:
                    emit_tiny_chain(s)

            def emit_tiny_chain(s):
                g_sb = wpool.tile([BDK, BDK], f32, name="g_sb", tag="g_sb",
                                  bufs=2)
                nc.vector.tensor_mul(g_sb[:], g_tiles[s][0:BDK, :], mk_t[:])
                m1_ps = pp.tile([BDK, K], f32, name="m1", tag="tiny", bufs=2)
                nc.tensor.matmul(m1_ps[:], g_sb[:], wq_t[:], start=True,
                                 stop=True)
                m1_sb = wpool.tile([BDK, K], f32, name="m1_sb", tag="m1_sb",
                                   bufs=2)
                nc.scalar.copy(m1_sb[:], m1_ps[:])
                st_ps = pp.tile([BDK, K], f32, name="st", tag="tiny", bufs=2)
                nc.tensor.matmul(st_ps[:], wk_t[:], m1_sb[:], start=True,
                                 stop=True)
                esl = slice(K * s, K * (s + 1))
                if have_bias:
                    eb_sb = wpool.tile([BDK, K], f32, name="eb_sb", tag="eb")
                    nc.vector.tensor_add(eb_sb[:], st_ps[:], corr_t[:, esl])
                    nc.scalar.activation(exp_all[:, esl], eb_sb[:], Exp)
                else:
                    nc.scalar.activation(exp_all[:, esl], st_ps[:], Exp)

            pend = []

            def emit_A_stack(s):
                if s == 2:
                    emit_const_dmas()
                for gi in range(n_grp):
                    pend.append((s, gi, emit_transposes(s, gi)))
                    # just-in-time prefetch, interleaved with compute emission
                    while state["loaded"] < min(s + 3, S):
                        load_xtile(state["loaded"])
                        state["loaded"] += 1
                    if len(pend) > 2:
                        ps, pgi, xt = pend.pop(0)
                        emit_gram(ps, pgi, xt)
                state["a_left"] -= 1

            def drain_pend():
                while pend:
                    ps, pgi, xt = pend.pop(0)
                    emit_gram(ps, pgi, xt)

            # --- phase C (per half): segment sums -> reciprocal ---
            def emit_phase_C(trange):
                seg_tiles = {}
                for t in trange:
                    idxs = [i for i, (ss, tt) in enumerate(used) if tt == t]
                    seg_ps = pp.tile([BDK + 1, BDK], f32, name=f"seg{t}",
                                     tag="gsb", bufs=2)
                    for n, i in enumerate(idxs):
                        s = used[i][0]
                        nc.tensor.matmul(
                            seg_ps[0:BDK, 0:K],
                            sel_t[:, BDK * i:BDK * (i + 1)],
                            exp_all[:, K * s:K * (s + 1)],
                            start=(n == 0), stop=(n == len(idxs) - 1),
                        )
                    seg_tiles[t] = seg_ps
                for t in trange:
                    seg_sb = wpool.tile([BDK, K], f32, name="seg_sb",
                                        tag="seg_sb")
                    nc.vector.tensor_scalar_max(
                        seg_sb[:], seg_tiles[t][0:BDK, 0:K], 1e-30)
                    nc.vector.reciprocal(inv_t[:, K * t:K * (t + 1)],
                                         seg_sb[:])

            # --- phase D: pipelined per stack ---
            state["nrelu"] = 0
            attn_tiles = {}

            def emit_attn_chain(s):
                idxs = [i for i, (ss, tt) in enumerate(used) if ss == s]
                invb_ps = pp.tile([BDK, K], f32, name="invb", tag="tiny",
                                  bufs=2)
                for n, i in enumerate(idxs):
                    t = used[i][1]
                    nc.tensor.matmul(
                        invb_ps[:],
                        selt_t[:, BDK * i:BDK * (i + 1)],
                        inv_t[:, K * t:K * (t + 1)],
                        start=(n == 0), stop=(n == len(idxs) - 1),
                    )
                attn_sb = wpool.tile([BDK, K], bf16, name="attn_sb",
                                     tag="attn_c", bufs=2)
                nc.vector.tensor_mul(attn_sb[:], exp_all[:, K * s:K * (s + 1)],
                                     invb_ps[:])
                # block-diag mask applied in one op via a 0-stride broadcast
                attn_bd = wpool.tile([BDK, BDK], bf16, name="attn_bd",
                                     tag="attn", bufs=2)
                attn_rep = attn_sb[:].unsqueeze(1).to_broadcast((BDK, BD, K))
                nc.vector.tensor_mul(attn_bd[:], attn_rep, mkb_t[:])
                attn_tiles[s] = attn_bd

            def emit_D_stack(s):
                b_ps = pp.tile([BDK + 1, BDK], f32, name="b_ps", tag="gsb",
                               bufs=2)
                nc.tensor.matmul(b_ps[:], wv_t[:], attn_tiles.pop(s)[:],
                                 start=True, stop=True)
                # pad B to 128 weight cols: enables PE fast weight load on
                # the output matmuls (extra PSUM rows are never read)
                b_sb = wpool.tile([BDK + 1, WPAD], bf16, name="b_sb", tag="B",
                                  bufs=2)
                nc.vector.tensor_add(b_sb[:, 0:BDK], b_ps[:], ia_t[:])
                nc.gpsimd.memset(b_sb[:, BDK:WPAD], 0.0)
                if s + 1 < S and (s + 1) not in phase_start:
                    emit_attn_chain(s + 1)  # overlaps this stack's matmuls

                osb = opool.tile([BDK, HW], bf16, name="osb", tag="osb",
                                 bufs=3)
                xt_ = x_tiles[s]
                tail = state["a_left"] == 0 and state["d_after_a"] >= 2
                if state["a_left"] == 0:
                    state["d_after_a"] += 1
                for oc in range(HW // O_CH):
                    sl = slice(O_CH * oc, O_CH * (oc + 1))
                    o_ps = pp.tile([WPAD, O_CH], f32, name="o_ps", tag="big",
                                   bufs=4)
                    nc.tensor.matmul(o_ps[:], b_sb[:], xt_[:, sl],
                                     start=True, stop=True)
                    on_act = (oc % 2 == 1) if tail else (oc % 4 == 3)
                    if on_act:
                        nc.scalar.activation(osb[:, sl], o_ps[0:BDK, :], Relu)
                    else:
                        nc.vector.tensor_scalar_max(osb[:, sl],
                                                    o_ps[0:BDK, :], 0.0)
                    state["nrelu"] += 1
                nc.gpsimd.dma_start(
                    y_d.ap()[:, s * HW:(s + 1) * HW], osb[:])

            # --- schedule: A(p0) | C0 | then per phase p: A(p+1) interleaved
            # stack-wise with D(p) | C(p+1) — loads and stores flow
            # continuously through the SWDGE ring ---
            state["interleave"] = False
            for s in range(cumS[0], cumS[1]):
                emit_A_stack(s)
            drain_pend()
            emit_phase_C(range(cumT[0], cumT[1]))
            emit_attn_chain(0)
            state["interleave"] = True
            for p in range(NP):
                nxtA = list(range(cumS[p + 1], cumS[p + 2])) \
                    if p + 1 < NP else []
                curD = list(range(cumS[p], cumS[p + 1]))
                for j in range(max(len(nxtA), len(curD))):
                    if j < len(nxtA):
                        emit_A_stack(nxtA[j])
                    if j < len(curD):
                        emit_D_stack(curD[j])
                if p + 1 < NP:
                    drain_pend()
                    emit_phase_C(range(cumT[p + 1], cumT[p + 2]))
                    emit_attn_chain(cumS[p + 1])

    nc.compile()
    return nc


def _get_compiled(S_list, T_list, have_bias: bool, used: tuple):
    key = (S_list, T_list, have_bias, used)
    if key not in _cache:
        _cache[key] = _build(S_list, T_list, have_bias, used)
    return _cache[key]


def _bd7(m: np.ndarray) -> np.ndarray:
    out = np.zeros((BDK, BDK), dtype=np.float32)
    for j in range(BD):
        out[K * j:K * (j + 1), K * j:K * (j + 1)] = m
    return out


def _plan(ids: np.ndarray):
    """Split persons into N_CORES contiguous chunks at imgid boundaries, each
    chunk into NP softmax-independent phases (again at group boundaries).
    All cores run the same compiled program padded to S_list stacks per
    phase, so the objective is minimizing sum(S_list) with several smallish
    phases (pipelining granularity): greedy furthest-reach per core."""
    change = np.flatnonzero(np.diff(ids)) + 1
    allb = np.concatenate([[0], change, [P_TOTAL]]).astype(np.int64)

    def plan_for(parts):
        bounds, midsl = [0], []
        for _ in range(N_CORES):
            a = bounds[-1]
            cur, cuts = a, []
            for sp in parts:
                cand = allb[(allb >= cur) & (allb <= cur + BD * sp)]
                cur = int(cand[-1])
                cuts.append(cur)
            bounds.append(cur)
            midsl.append([c - a for c in cuts[:-1]])
            if cur == P_TOTAL:
                break
        while len(bounds) < N_CORES + 1:
            bounds.append(bounds[-1])
            midsl.append([0] * (len(parts) - 1))
        return (bounds, midsl) if bounds[-1] == P_TOTAL else None

    def comps(total, np_):
        if np_ == 1:
            yield (total,)
            return
        for first in range(1, total - np_ + 2):
            for rest in comps(total - first, np_ - 1):
                yield (first,) + rest

    Smin = math.ceil(P_TOTAL / (N_CORES * BD))
    cands = []
    for Stot in range(Smin, Smin + 4):
        for np_ in (5, 4, 3, 2):
            if np_ > Stot:
                continue
            for c in comps(Stot, np_):
                # prefer: few total stacks, small first+last phases (serial
                # exposure), many phases, balanced
                pen = (Stot, c[0] + c[-1], -np_,
                       max(c) - min(c))
                cands.append((pen, c))
    cands.sort(key=lambda x: x[0])
    best = None
    for _, parts in cands:
        r = plan_for(parts)
        if r is not None:
            best = (parts,) + r
            break
    assert best is not None
    S_list, bounds, midsl = best
    NP = len(S_list)

    g_max = [1] * NP
    for ci in range(N_CORES):
        a, b = bounds[ci], bounds[ci + 1]
        cuts = [0] + midsl[ci] + [b - a]
        for p in range(NP):
            lo, hi = a + cuts[p], a + cuts[p + 1]
            if hi > lo:
                g_max[p] = max(g_max[p], len(np.unique(ids[lo:hi])))
    T_list = tuple(math.ceil((g + 1) / BD) for g in g_max)
    return bounds, midsl, tuple(S_list), T_list


def _prepare(inputs: dict):
    import ml_dtypes
    nbf16 = ml_dtypes.bfloat16

    x = np.asarray(inputs["kpt_feat"], dtype=np.float32).reshape(
        P_TOTAL, K, HW)
    ids = np.asarray(inputs["imgid"]).astype(np.int64)
    Wq = np.asarray(inputs["Wq"], np.float32)
    Wk = np.asarray(inputs["Wk"], np.float32)
    Wv = np.asarray(inputs["Wv"], np.float32)
    bq = np.asarray(inputs["bq"], np.float32)
    bk = np.asarray(inputs["bk"], np.float32)
    bv = np.asarray(inputs["bv"], np.float32)

    bounds, midsl, S_list, T_list = _plan(ids)
    NP = len(S_list)
    S = sum(S_list)
    T = sum(T_list)
    cumS = [0]
    for sp in S_list:
        cumS.append(cumS[-1] + sp)
    cumT = [0]
    for tp in T_list:
        cumT.append(cumT[-1] + tp)
    P_pad = S * BD

    wq_col = np.zeros((BDK, K), np.float32)
    for j in range(BD):
        wq_col[K * j:K * (j + 1), :] = Wq.T / NORM
    wkt_bd = _bd7(Wk.T.astype(np.float32))
    wv_aug = np.zeros((BDK, BDK + 1), np.float32)
    wv_aug[:, :BDK] = _bd7(Wv)
    for j in range(BD):
        wv_aug[K * j:K * (j + 1), BDK] = bv
    wv_aug = wv_aug.astype(nbf16)
    id119 = np.eye(BDK, dtype=np.float32).astype(nbf16)
    iaug = np.zeros((BDK + 1, BDK), np.float32)
    iaug[:BDK, :BDK] = np.eye(BDK, dtype=np.float32)
    bdmask = _bd7(np.ones((K, K), np.float32))

    have_bias = bool(np.any(bq) or np.any(bk))
    if have_bias:
        xsum = x.sum(axis=2)
        qx = xsum @ Wq.T
        kx = xsum @ Wk.T
        corr_all = (bk[None, :, None] * qx[:, None, :]
                    + bq[None, None, :] * kx[:, :, None]
                    + HW * (bq[None, None, :] * bk[None, :, None])) / NORM
        corr_all = corr_all.astype(np.float32)  # [P, m, i]
    else:
        corr_all = None

    xb = x.astype(nbf16)

    # person permutation (two padded halves) + selector tensors per core
    eye = np.eye(K, dtype=np.float32)
    sels = []
    newpos_all = []
    used_set = set()
    for ci in range(N_CORES):
        a, b = bounds[ci], bounds[ci + 1]
        pc = b - a
        cuts = [0] + midsl[ci] + [pc]
        # newpos[old person idx within core] -> padded position; phase p's
        # persons start at padded position BD*cumS[p]
        newpos = np.concatenate(
            [BD * cumS[p] + np.arange(cuts[p + 1] - cuts[p])
             for p in range(NP)])
        newpos_all.append(newpos)
        # slots per padded position: phase p groups at 7*cumT[p]..;
        # padding persons go to the phase's dummy slot
        slots = np.full((P_pad,), 0, np.int64)
        for p in range(NP):
            slots[BD * cumS[p]:BD * cumS[p + 1]] = 7 * cumT[p + 1] - 1
            lo, hi = a + cuts[p], a + cuts[p + 1]
            if hi > lo:
                _, lg = np.unique(ids[lo:hi], return_inverse=True)
                slots[BD * cumS[p]:BD * cumS[p] + (hi - lo)] = \
                    7 * cumT[p] + lg
        sel = np.zeros((S, T, BDK, BDK), np.float32)
        for s in range(S):
            for j in range(BD):
                g = slots[BD * s + j]
                t, lgi = divmod(g, BD)
                sel[s, t, K * j:K * (j + 1), K * lgi:K * (lgi + 1)] = eye
                used_set.add((s, t))
        sels.append(sel)
    used = tuple(sorted(used_set))

    in_maps = []
    for ci in range(N_CORES):
        a, b = bounds[ci], bounds[ci + 1]
        pc = b - a
        # partition-major x: [120, S*HW]; row 119 = ones (residual fold)
        np_ = newpos_all[ci]
        rows = np.zeros((P_pad, K, HW), dtype=nbf16)
        if pc:
            rows[np_] = xb[a:b]
        rows = rows.reshape(P_pad * K, HW)
        arr3 = np.zeros((S, BDK + 1, HW), dtype=nbf16)
        arr3[:, :BDK] = rows.reshape(S, BDK, HW)
        arr3[:, BDK] = 1.0
        xs = np.ascontiguousarray(
            arr3.transpose(1, 0, 2).reshape(BDK + 1, S * HW))
        sel = sels[ci]
        su = np.stack([sel[s, t] for (s, t) in used])  # [U, 119, 119]
        sel_pack = su.transpose(1, 0, 2).reshape(BDK, len(used) * BDK)
        selt_pack = su.transpose(2, 0, 1).reshape(BDK, len(used) * BDK)
        m = {
            "x": xs,
            "wq_col": wq_col,
            "wkt_bd": wkt_bd,
            "wv_aug": wv_aug,
            "id119": id119,
            "iaug": iaug,
            "bdmask": bdmask,
            "bdmaskb": bdmask.astype(nbf16),
            "sel": np.ascontiguousarray(sel_pack).astype(nbf16),
            "selT": np.ascontiguousarray(selt_pack).astype(nbf16),
        }
        if have_bias:
            corr_col = np.zeros((BDK, K * S), np.float32)
            if pc:
                cpad = np.zeros((P_pad, K, K), np.float32)
                cpad[np_] = corr_all[a:b]
                for s in range(S):
                    for j in range(BD):
                        corr_col[K * j:K * (j + 1), K * s:K * (s + 1)] = \
                            cpad[BD * s + j]
            m["corr_col"] = corr_col
        in_maps.append(m)
    return in_maps, bounds, newpos_all, (S_list, T_list), have_bias, used


def _gather(results, bounds, newpos_all, S):
    out = np.empty((P_TOTAL, K, 64, 64), dtype=np.float32)
    for ci in range(N_CORES):
        a, b = bounds[ci], bounds[ci + 1]
        pc = b - a
        if pc:
            y = np.asarray(results[ci]["y"], dtype=np.float32)  # [119, S*HW]
            y = y.reshape(BDK, S, HW).transpose(1, 0, 2).reshape(
                S * BD, K, 64, 64)
            out[a:b] = y[newpos_all[ci]]
    return out


def _run(inputs: dict, trace: bool = False):
    _ensure_path()
    from concourse.bass_utils import run_bass_kernel_spmd

    in_maps, bounds, newpos_all, (S_list, T_list), have_bias, used = \
        _prepare(inputs)
    nc = _get_compiled(S_list, T_list, have_bias, used)
    res = run_bass_kernel_spmd(nc, in_maps, list(range(N_CORES)), trace=trace)
    return _gather(res.results, bounds, newpos_all, sum(S_list)), res


def kernel(**inputs) -> np.ndarray:
    out, _ = _run(inputs, trace=False)
    return out
